# revision 27
# baseline (speedup 1.0000x reference)
"""Trainium2 Bass kernel for nn_Attention_23424751632639.

Computation (per (b,h)):  out = tril_strict(rope(Q) @ rope(Q).T / sqrt(N)) @ V
Chunked linear attention (exact):
  out_c = QR_c @ M_c  +  strict_mask(QR_c @ QR_c^T) @ V_c
  M_{c+1} = M_c + QR_c^T @ V_c
with QR = rope(Q) * N**-0.25 (scale folded into cos/sin tables).

Design:
- Inputs loaded via SWDGE cast-DMA (fp32 HBM -> bf16 SBUF) from the gpsimd
  queue: zero engine cost for the cast, bf16 rope at DVE 2x mode.
- rope = q*CC + swapview(q)*SS, where swapview is a reversed-pair AP on the
  multiply input (no separate swap op).
- Two heads processed side by side ("head pair"): their QR^T strips live at
  partitions 0-63 / 64-127, so K=64 matmuls (scores P, inter) run
  concurrently in different PE row groups, and PSUM->SBUF copies serve both
  heads in one [128, *] op.
- Per chunk-pair slot: one [128,512] PSUM score tile (4 diag blocks),
  bridged to SBUF bf16 by a single masked DVE tensor_tensor.
- M state accumulates in PSUM (one [128,64] tile = both heads); per-chunk
  snapshots to SBUF bf16 on ACT feed the inter matmuls.
- Output written bf16 in a partition-major layout (big DMA runs), host
  unpermutes and casts to fp32.
Sharding: B*H = 32 (b,h) pairs -> 4 per core across 8 cores; no collectives.
"""

import math
import sys

import numpy as np

if "/opt/trn_rl_repo" not in sys.path:
    sys.path.insert(0, "/opt/trn_rl_repo")

B, H, T, N = 2, 16, 4096, 64
THETA = 2.0 ** 16
NCORES = 8
HPC = (B * H) // NCORES    # heads per core
CH = T // 128              # chunks per head
NHP = HPC // 2             # head pairs per core


def _host_tables():
    """Scaled RoPE tables, bf16, [128, CH*64]: partition p, (chunk c,
    feature n); table row t = c*128 + p.  The pair-swap is pre-applied to
    ss (ss_sw[2m] = -sin[2m+1-pair partner]... see emit_rope): we store the
    tables for the split stride-2 multiplies: css / csc handled in kernel.
    """
    import ml_dtypes

    n = np.arange(N, dtype=np.float64)
    tq = np.floor(n / 2.0) * 2.0
    freqs = 1.0 / (THETA ** (tq / N)) / (2.0 * math.pi)   # [N]
    t = np.arange(T, dtype=np.float64)[:, None]
    ang = ((t * freqs[None, :]) % 1.0) * (2.0 * math.pi)  # [T, N]
    scale = float(N) ** -0.25
    cc = (np.cos(ang) * scale).astype(np.float32)
    ss = (np.sin(ang) * scale).astype(np.float32)
    ss[:, 0::2] *= -1.0
    # [T, N] -> [c, p, n] -> [p, c, n]
    cc = cc.reshape(CH, 128, N).transpose(1, 0, 2).reshape(128, CH * N)
    ss = ss.reshape(CH, 128, N).transpose(1, 0, 2).reshape(128, CH * N)
    return (np.ascontiguousarray(cc.astype(ml_dtypes.bfloat16)),
            np.ascontiguousarray(ss.astype(ml_dtypes.bfloat16)))


def _mask_mu():
    """[128, 512] bf16: strict-upper keep mask (s < t), tiled 4x."""
    import ml_dtypes

    m = np.triu(np.ones((128, 128), dtype=np.float32), k=1)
    return np.ascontiguousarray(
        np.tile(m, (1, 4)).astype(ml_dtypes.bfloat16))


def _identity():
    import ml_dtypes

    return np.eye(128, dtype=ml_dtypes.bfloat16)


def build_program():
    import concourse.mybir as mybir
    import concourse.tile as tile
    from concourse import bacc

    f32 = mybir.dt.float32
    bf = mybir.dt.bfloat16

    nc = bacc.Bacc(None, target_bir_lowering=False)
    q = nc.dram_tensor("q", [HPC, T, N], f32, kind="ExternalInput")
    v = nc.dram_tensor("v", [HPC, T, N], f32, kind="ExternalInput")
    cc2 = nc.dram_tensor("cc2", [128, CH * N], bf, kind="ExternalInput")
    ss2 = nc.dram_tensor("ss2", [128, CH * N], bf, kind="ExternalInput")
    mu = nc.dram_tensor("mu", [128, 512], bf, kind="ExternalInput")
    ident = nc.dram_tensor("ident", [128, 128], bf, kind="ExternalInput")
    o = nc.dram_tensor("o", [128, HPC * CH * N], bf, kind="ExternalOutput")

    with tile.TileContext(nc) as tc:
        with (
            tc.tile_pool(name="const", bufs=1) as constp,
            tc.tile_pool(name="head", bufs=1) as headp,
            tc.tile_pool(name="rope", bufs=3) as ropep,
            tc.tile_pool(name="small", bufs=6) as smallp,
            tc.tile_pool(name="pb", bufs=3) as pbp,
            tc.tile_pool(name="ost", bufs=3) as ostp,
            tc.tile_pool(name="trp", bufs=2, space="PSUM") as trpp,
            tc.tile_pool(name="pp", bufs=3, space="PSUM") as ppp,
            tc.tile_pool(name="op", bufs=2, space="PSUM") as opp,
            tc.tile_pool(name="mp", bufs=1, space="PSUM") as mpp,
        ):
            cc_sb = constp.tile([128, CH * N], bf)
            ss_sb = constp.tile([128, CH * N], bf)
            mu_sb = constp.tile([128, 512], bf)
            id_sb = constp.tile([128, 128], bf)
            nc.sync.dma_start(cc_sb[:], cc2[:])
            nc.sync.dma_start(ss_sb[:], ss2[:])
            nc.sync.dma_start(mu_sb[:], mu[:])
            nc.sync.dma_start(id_sb[:], ident[:])

            # persistent tiles: per (hp, hi) contiguous [128, CH*64]
            qi = {}   # bf16 input (c, n)
            qr = {}   # rope output, same layout
            qrt = {}  # per hp [64, CH*256]: chunk strips (h0 | h1) on
                      # partitions 0-63 (base-64 matmul operands hit the
                      # quadrant-3 HW bug, so everything stays at base 0)
            vsb = {}
            for hp in range(NHP):
                qrt[hp] = headp.tile([64, CH * 256], bf, name=f"qrt{hp}",
                                     tag=f"qrt{hp}")
                for hi in range(2):
                    qi[(hp, hi)] = headp.tile(
                        [128, CH * N], bf, name=f"qi{hp}{hi}",
                        tag=f"qi{hp}{hi}")
                    qr[(hp, hi)] = headp.tile(
                        [128, CH * N], bf, name=f"qr{hp}{hi}",
                        tag=f"qr{hp}{hi}")
                    vsb[(hp, hi)] = headp.tile(
                        [128, CH * N], bf, name=f"v{hp}{hi}",
                        tag=f"v{hp}{hi}")

            # input loads: SWDGE cast fp32 -> bf16, small first pieces for a
            # fast pipeline start, queued in consumption order
            def load_q(hp, hi, a, b):
                dst = qi[(hp, hi)].rearrange("p (c n) -> p c n", c=CH)[
                    :, a:b]
                src = q[2 * hp + hi].rearrange("(c p) n -> p c n", p=128)[
                    :, a:b]
                nc.gpsimd.dma_start(dst, src)

            def load_v(hp, hi, a, b):
                dst = vsb[(hp, hi)].rearrange("p (c n) -> p c n", c=CH)[
                    :, a:b]
                src = v[2 * hp + hi].rearrange("(c p) n -> p c n", p=128)[
                    :, a:b]
                nc.gpsimd.dma_start(dst, src)

            # hp0 in pieces for fast pipeline start, hp1 whole-head
            for hi in range(2):
                load_q(0, hi, 0, 8)
            for hi in range(2):
                load_v(0, hi, 0, 8)
            for hi in range(2):
                load_q(0, hi, 8, CH)
            for hi in range(2):
                load_v(0, hi, 8, CH)
            for hi in range(2):
                load_q(1, hi, 0, CH)
            for hi in range(2):
                load_v(1, hi, 0, CH)

            RS = 4  # rope stage size in chunks
            n_stages = CH // RS

            def emit_rope(hp, hi, st):
                a = st * RS
                sl = slice(a * N, (a + RS) * N)
                psl = slice(a * N // 2, (a + RS) * N // 2)
                t1 = ropep.tile([128, RS * N], bf, name="t1", tag="t1")
                t2 = ropep.tile([128, RS * N], bf, name="t2", tag="t2")
                qq = qi[(hp, hi)]
                nc.vector.tensor_mul(t1[:], qq[:, sl], cc_sb[:, sl])
                # swap(q)*ss as two stride-2 multiplies (positive strides)
                q3 = qq.rearrange("p (x o) -> p x o", o=2)[:, psl]
                s3 = ss_sb.rearrange("p (x o) -> p x o", o=2)[:, psl]
                t3 = t2.rearrange("p (x o) -> p x o", o=2)
                nc.vector.tensor_mul(t3[:, :, 0], q3[:, :, 1], s3[:, :, 0])
                nc.vector.tensor_mul(t3[:, :, 1], q3[:, :, 0], s3[:, :, 1])
                nc.vector.tensor_add(qr[(hp, hi)][:, sl], t1[:], t2[:])

            # rope schedule: hp0 stage 0 in prologue; hp0 stages 1..7 at
            # global slots 0..6; hp1 stages 0..7 at global slots 5..12
            rope_sched = {}
            for st in range(1, n_stages):
                rope_sched.setdefault(st - 1, []).append((0, st))
            for st in range(n_stages):
                rope_sched.setdefault(5 + st, []).append((1, st))

            emit_rope(0, 0, 0)
            emit_rope(0, 1, 0)

            m_ps = {}

            NSLOT = 16

            def strip(hp, hi, c):
                return qrt[hp][:, c * 256 + hi * 128:
                               c * 256 + (hi + 1) * 128]

            def emit_transposes(hp, s):
                # all 4 strips of a slot into one PSUM tile, one ACT copy
                c0 = 2 * s
                trp = trpp.tile([64, 512], bf, tag="trp")
                for k in range(2):
                    for hi in range(2):
                        nc.tensor.transpose(
                            trp[:, (2 * k + hi) * 128:(2 * k + hi + 1) * 128],
                            qr[(hp, hi)][:, (c0 + k) * N:(c0 + k + 1) * N],
                            id_sb[:])
                nc.scalar.copy(
                    qrt[hp][:, c0 * 256:(c0 + 2) * 256], trp[:])

            snaps = {}   # (hp, s) -> (snap_a or None, snap_b)
            pend = {}    # g -> (hp, s, p_sb, out_ps)
            snap_a_prev = {0: None, 1: None}

            def emit_B(g_prev):
                hp, s, p_sb, out_ps = pend.pop(g_prev)
                c0, c1 = 2 * s, 2 * s + 1
                snap_a, snap_b = snaps.pop((hp, s))

                def osl(hi, k):
                    return slice((2 * hi + k) * 64, (2 * hi + k + 1) * 64)

                n_out_mm = 8 if snap_a is not None else 6
                mm_i = [0]

                def out_mm(dst, lhsT, rhs):
                    i = mm_i[0]
                    mm_i[0] += 1
                    nc.tensor.matmul(dst, lhsT, rhs, start=(i == 0),
                                     stop=(i == n_out_mm - 1))

                if snap_a is not None:
                    for hi in range(2):
                        out_mm(out_ps[:, osl(hi, 0)], strip(hp, hi, c0),
                               snap_a[:, hi * 64:(hi + 1) * 64])
                for hi in range(2):
                    out_mm(out_ps[:, osl(hi, 1)], strip(hp, hi, c1),
                           snap_b[:, hi * 64:(hi + 1) * 64])
                for k, c in ((0, c0), (1, c1)):
                    for hi in range(2):
                        out_mm(
                            out_ps[:, osl(hi, k)],
                            p_sb[:, (2 * k + hi) * 128:(2 * k + hi + 1) * 128],
                            vsb[(hp, hi)][:, c * N:(c + 1) * N])

                # stage out bf16 and store (one DMA per slot)
                ot = ostp.tile([128, 256], bf, name="ot", tag="ost")
                nc.scalar.copy(ot[:], out_ps[:])
                dst = o.rearrange("p (hh c x) -> p hh c x",
                                  hh=HPC, c=CH // 2)[:, 2 * hp:2 * hp + 2, s]
                src = ot.rearrange("p (h x) -> p h x", h=2)
                nc.sync.dma_start(dst, src)

            # HAM warmup: ~4us of dep-free back-to-back matmuls so the PE
            # clock ungates to 2.4 GHz before the main loop
            wu = mpp.tile([128, 512], f32, name="wu", tag="m", bufs=1)
            for i in range(12):
                nc.tensor.matmul(wu[:], id_sb[:], cc_sb[:, 0:512],
                                 start=True, stop=True)

            # prologue: transposes for slot 0
            emit_transposes(0, 0)

            for g in range(2 * NSLOT):
                hp, s = g // NSLOT, g % NSLOT
                c0, c1 = 2 * s, 2 * s + 1
                my_snap_a = snap_a_prev[hp]
                if s == 0:
                    m_ps[hp] = mpp.tile([64, 2 * N], f32, name=f"m{hp}",
                                        tag="m", bufs=1)

                # transposes + qrt copy for the NEXT slot (PE + ACT ahead)
                if g + 1 < 2 * NSLOT:
                    nhp, ns = (g + 1) // NSLOT, (g + 1) % NSLOT
                    emit_transposes(nhp, ns)

                # scores for this slot
                p_ps = ppp.tile([128, 512], f32, tag="pp")
                for k, c in ((0, c0), (1, c1)):
                    for hi in range(2):
                        nc.tensor.matmul(
                            p_ps[:, (2 * k + hi) * 128:(2 * k + hi + 1) * 128],
                            strip(hp, hi, c), strip(hp, hi, c),
                            start=True, stop=True)

                # state update with c0 (waits snap_a, copied last slot)
                for hi in range(2):
                    nc.tensor.matmul(
                        m_ps[hp][:, hi * 64:(hi + 1) * 64],
                        qr[(hp, hi)][:, c0 * N:(c0 + 1) * N],
                        vsb[(hp, hi)][:, c0 * N:(c0 + 1) * N],
                        start=(c0 == 0 and hi == 0), stop=False,
                        skip_group_check=True)

                # snapshot M<c1 (after states c0, before states c1)
                snap_b = smallp.tile([64, 2 * N], bf, name="snb", tag="snap")
                nc.scalar.copy(snap_b[:], m_ps[hp][:])

                # B-stage of the previous slot fills PE while snap_b copies
                if g - 1 in pend:
                    emit_B(g - 1)

                # state update with c1 (skip for the last chunk: never read)
                if c1 < CH - 1:
                    for hi in range(2):
                        nc.tensor.matmul(
                            m_ps[hp][:, hi * 64:(hi + 1) * 64],
                            qr[(hp, hi)][:, c1 * N:(c1 + 1) * N],
                            vsb[(hp, hi)][:, c1 * N:(c1 + 1) * N],
                            start=False, stop=(c1 == CH - 2),
                            skip_group_check=True)

                # snapshot M<c0 of the NEXT slot (same hp only)
                snap_a = None
                if s + 1 < NSLOT:
                    snap_a = smallp.tile([64, 2 * N], bf, name="sna",
                                         tag="snap")
                    nc.scalar.copy(snap_a[:], m_ps[hp][:])

                # bridge PSUM -> SBUF with the strict mask folded in
                p_sb = pbp.tile([128, 512], bf, name="psb", tag="psb")
                nc.vector.tensor_mul(p_sb[:], p_ps[:], mu_sb[:])

                # record for B-stage next slot
                out_ps = opp.tile([128, 256], f32, name="outp", tag="outp")
                snaps[(hp, s)] = (my_snap_a, snap_b)
                snap_a_prev[hp] = snap_a
                pend[g] = (hp, s, p_sb, out_ps)

                for (rhp, st) in rope_sched.get(g, []):
                    emit_rope(rhp, 0, st)
                    emit_rope(rhp, 1, st)

            emit_B(2 * NSLOT - 1)

    nc.compile()
    return nc


_CACHE = {}


def _get_program():
    if "nc" not in _CACHE:
        _CACHE["nc"] = build_program()
    return _CACHE["nc"]


def _in_maps(Q, V):
    Q = np.ascontiguousarray(np.asarray(Q), dtype=np.float32)
    V = np.ascontiguousarray(np.asarray(V), dtype=np.float32)
    qf = Q.reshape(NCORES, HPC, T, N)
    vf = V.reshape(NCORES, HPC, T, N)
    cc2, ss2 = _host_tables()
    mu = _mask_mu()
    ident = _identity()
    return [
        {"q": qf[i], "v": vf[i], "cc2": cc2, "ss2": ss2, "mu": mu,
         "ident": ident}
        for i in range(NCORES)
    ]


def kernel(Q, V):
    from concourse.bass_utils import run_bass_kernel_spmd

    nc = _get_program()
    res = run_bass_kernel_spmd(nc, _in_maps(Q, V),
                               core_ids=list(range(NCORES)))
    # o: [128, HPC, CH, N] bf16, partition-major -> [HPC, T, N] fp32
    out = np.stack(
        [np.asarray(r["o"]).astype(np.float32)
         .reshape(128, HPC, CH, N).transpose(1, 2, 0, 3)
         .reshape(HPC, T, N)
         for r in res.results], axis=0)
    return out.reshape(B, H, T, N)


# revision 30
# speedup vs baseline: 1.0267x; 1.0267x over previous
"""Trainium2 Bass kernel for nn_Attention_23424751632639.

Computation (per (b,h)):  out = tril_strict(rope(Q) @ rope(Q).T / sqrt(N)) @ V
Chunked linear attention (exact):
  out_c = QR_c @ M_c  +  strict_mask(QR_c @ QR_c^T) @ V_c
  M_{c+1} = M_c + QR_c^T @ V_c
with QR = rope(Q) * N**-0.25 (scale folded into cos/sin tables).

Design:
- Inputs loaded via SWDGE cast-DMA (fp32 HBM -> bf16 SBUF) from the gpsimd
  queue: zero engine cost for the cast, bf16 rope at DVE 2x mode.
- rope = q*CC + swapview(q)*SS, where swapview is a reversed-pair AP on the
  multiply input (no separate swap op).
- Two heads processed side by side ("head pair"): their QR^T strips live at
  partitions 0-63 / 64-127, so K=64 matmuls (scores P, inter) run
  concurrently in different PE row groups, and PSUM->SBUF copies serve both
  heads in one [128, *] op.
- Per chunk-pair slot: one [128,512] PSUM score tile (4 diag blocks),
  bridged to SBUF bf16 by a single masked DVE tensor_tensor.
- M state accumulates in PSUM (one [128,64] tile = both heads); per-chunk
  snapshots to SBUF bf16 on ACT feed the inter matmuls.
- Output written bf16 in a partition-major layout (big DMA runs), host
  unpermutes and casts to fp32.
Sharding: B*H = 32 (b,h) pairs -> 4 per core across 8 cores; no collectives.
"""

import math
import sys

import numpy as np

if "/opt/trn_rl_repo" not in sys.path:
    sys.path.insert(0, "/opt/trn_rl_repo")

B, H, T, N = 2, 16, 4096, 64
THETA = 2.0 ** 16
NCORES = 8
HPC = (B * H) // NCORES    # heads per core
CH = T // 128              # chunks per head
NHP = HPC // 2             # head pairs per core


def _host_tables():
    """Scaled RoPE tables, bf16, [128, CH*64]: partition p, (chunk c,
    feature n); table row t = c*128 + p.  The pair-swap is pre-applied to
    ss (ss_sw[2m] = -sin[2m+1-pair partner]... see emit_rope): we store the
    tables for the split stride-2 multiplies: css / csc handled in kernel.
    """
    import ml_dtypes

    n = np.arange(N, dtype=np.float64)
    tq = np.floor(n / 2.0) * 2.0
    freqs = 1.0 / (THETA ** (tq / N)) / (2.0 * math.pi)   # [N]
    t = np.arange(T, dtype=np.float64)[:, None]
    ang = ((t * freqs[None, :]) % 1.0) * (2.0 * math.pi)  # [T, N]
    scale = float(N) ** -0.25
    cc = (np.cos(ang) * scale).astype(np.float32)
    ss = (np.sin(ang) * scale).astype(np.float32)
    ss[:, 0::2] *= -1.0
    # [T, N] -> [c, p, n] -> [p, c, n]
    cc = cc.reshape(CH, 128, N).transpose(1, 0, 2).reshape(128, CH * N)
    ss = ss.reshape(CH, 128, N).transpose(1, 0, 2).reshape(128, CH * N)
    return (np.ascontiguousarray(cc.astype(ml_dtypes.bfloat16)),
            np.ascontiguousarray(ss.astype(ml_dtypes.bfloat16)))


def _mask_mu():
    """[128, 512] bf16: strict-upper keep mask (s < t), tiled 4x."""
    import ml_dtypes

    m = np.triu(np.ones((128, 128), dtype=np.float32), k=1)
    return np.ascontiguousarray(
        np.tile(m, (1, 4)).astype(ml_dtypes.bfloat16))


def _identity():
    import ml_dtypes

    return np.eye(128, dtype=ml_dtypes.bfloat16)


def build_program():
    import concourse.mybir as mybir
    import concourse.tile as tile
    from concourse import bacc

    f32 = mybir.dt.float32
    bf = mybir.dt.bfloat16

    nc = bacc.Bacc(None, target_bir_lowering=False)
    q = nc.dram_tensor("q", [HPC, T, N], f32, kind="ExternalInput")
    v = nc.dram_tensor("v", [HPC, T, N], f32, kind="ExternalInput")
    cc2 = nc.dram_tensor("cc2", [128, CH * N], bf, kind="ExternalInput")
    ss2 = nc.dram_tensor("ss2", [128, CH * N], bf, kind="ExternalInput")
    mu = nc.dram_tensor("mu", [128, 512], bf, kind="ExternalInput")
    ident = nc.dram_tensor("ident", [128, 128], bf, kind="ExternalInput")
    o = nc.dram_tensor("o", [128, HPC * CH * N], bf, kind="ExternalOutput")

    with tile.TileContext(nc) as tc:
        with (
            tc.tile_pool(name="const", bufs=1) as constp,
            tc.tile_pool(name="head", bufs=1) as headp,
            tc.tile_pool(name="rope", bufs=3) as ropep,
            tc.tile_pool(name="small", bufs=6) as smallp,
            tc.tile_pool(name="pb", bufs=3) as pbp,
            tc.tile_pool(name="ost", bufs=3) as ostp,
            tc.tile_pool(name="trp", bufs=2, space="PSUM") as trpp,
            tc.tile_pool(name="pp", bufs=2, space="PSUM") as ppp,
            tc.tile_pool(name="op", bufs=2, space="PSUM") as opp,
            tc.tile_pool(name="mp", bufs=1, space="PSUM") as mpp,
            tc.tile_pool(name="wup", bufs=1, space="PSUM") as wupp,
        ):
            cc_sb = constp.tile([128, CH * N], bf)
            ss_sb = constp.tile([128, CH * N], bf)
            mu_sb = constp.tile([128, 512], bf)
            id_sb = constp.tile([128, 128], bf)
            nc.sync.dma_start(cc_sb[:], cc2[:])
            nc.sync.dma_start(ss_sb[:], ss2[:])
            nc.sync.dma_start(mu_sb[:], mu[:])
            nc.sync.dma_start(id_sb[:], ident[:])

            # persistent tiles: per (hp, hi) contiguous [128, CH*64]
            qi = {}   # bf16 input (c, n)
            qr = {}   # rope output, same layout
            qrt = {}  # per hp [64, CH*256]: chunk strips (h0 | h1) on
                      # partitions 0-63 (base-64 matmul operands hit the
                      # quadrant-3 HW bug, so everything stays at base 0)
            vsb = {}
            for hp in range(NHP):
                qrt[hp] = headp.tile([64, CH * 256], bf, name=f"qrt{hp}",
                                     tag=f"qrt{hp}")
                for hi in range(2):
                    qi[(hp, hi)] = headp.tile(
                        [128, CH * N], bf, name=f"qi{hp}{hi}",
                        tag=f"qi{hp}{hi}")
                    qr[(hp, hi)] = headp.tile(
                        [128, CH * N], bf, name=f"qr{hp}{hi}",
                        tag=f"qr{hp}{hi}")
                    vsb[(hp, hi)] = headp.tile(
                        [128, CH * N], bf, name=f"v{hp}{hi}",
                        tag=f"v{hp}{hi}")

            # input loads: SWDGE cast fp32 -> bf16, small first pieces for a
            # fast pipeline start, queued in consumption order
            def load_q(hp, hi, a, b):
                dst = qi[(hp, hi)].rearrange("p (c n) -> p c n", c=CH)[
                    :, a:b]
                src = q[2 * hp + hi].rearrange("(c p) n -> p c n", p=128)[
                    :, a:b]
                nc.gpsimd.dma_start(dst, src)

            def load_v(hp, hi, a, b):
                dst = vsb[(hp, hi)].rearrange("p (c n) -> p c n", c=CH)[
                    :, a:b]
                src = v[2 * hp + hi].rearrange("(c p) n -> p c n", p=128)[
                    :, a:b]
                nc.gpsimd.dma_start(dst, src)

            # hp0 in pieces for fast pipeline start, hp1 whole-head
            for hi in range(2):
                load_q(0, hi, 0, 8)
            for hi in range(2):
                load_v(0, hi, 0, 8)
            for hi in range(2):
                load_q(0, hi, 8, CH)
            for hi in range(2):
                load_v(0, hi, 8, CH)
            for hi in range(2):
                load_q(1, hi, 0, CH)
            for hi in range(2):
                load_v(1, hi, 0, CH)

            RS = 4  # rope stage size in chunks
            n_stages = CH // RS

            def emit_rope(hp, hi, st):
                a = st * RS
                sl = slice(a * N, (a + RS) * N)
                psl = slice(a * N // 2, (a + RS) * N // 2)
                t1 = ropep.tile([128, RS * N], bf, name="t1", tag="t1")
                t2 = ropep.tile([128, RS * N], bf, name="t2", tag="t2")
                qq = qi[(hp, hi)]
                nc.vector.tensor_mul(t1[:], qq[:, sl], cc_sb[:, sl])
                # swap(q)*ss as two stride-2 multiplies (positive strides)
                q3 = qq.rearrange("p (x o) -> p x o", o=2)[:, psl]
                s3 = ss_sb.rearrange("p (x o) -> p x o", o=2)[:, psl]
                t3 = t2.rearrange("p (x o) -> p x o", o=2)
                nc.vector.tensor_mul(t3[:, :, 0], q3[:, :, 1], s3[:, :, 0])
                nc.vector.tensor_mul(t3[:, :, 1], q3[:, :, 0], s3[:, :, 1])
                nc.vector.tensor_add(qr[(hp, hi)][:, sl], t1[:], t2[:])

            # rope schedule: hp0 stage 0 in prologue; hp0 stages 1..7 at
            # global slots 0..6; hp1 stages 0..7 at global slots 5..12
            rope_sched = {}
            for st in range(1, n_stages):
                rope_sched.setdefault(st - 1, []).append((0, st))
            for st in range(n_stages):
                rope_sched.setdefault(5 + st, []).append((1, st))

            emit_rope(0, 0, 0)
            emit_rope(0, 1, 0)

            m_ps = {}

            NSLOT = 16

            def strip(hp, hi, c):
                return qrt[hp][:, c * 256 + hi * 128:
                               c * 256 + (hi + 1) * 128]

            def emit_transposes(hp, s):
                # all 4 strips of a slot into one PSUM tile, one ACT copy
                c0 = 2 * s
                trp = trpp.tile([64, 512], bf, tag="trp")
                for k in range(2):
                    for hi in range(2):
                        nc.tensor.transpose(
                            trp[:, (2 * k + hi) * 128:(2 * k + hi + 1) * 128],
                            qr[(hp, hi)][:, (c0 + k) * N:(c0 + k + 1) * N],
                            id_sb[:])
                nc.scalar.copy(
                    qrt[hp][:, c0 * 256:(c0 + 2) * 256], trp[:])

            snaps = {}   # (hp, s) -> (snap_a or None, snap_b)
            pend = {}    # g -> (hp, s, p_sb, out_ps)
            snap_a_prev = {0: None, 1: None}

            def emit_B(g_prev):
                hp, s, p_sb, out_ps = pend.pop(g_prev)
                c0, c1 = 2 * s, 2 * s + 1
                snap_a, snap_b = snaps.pop((hp, s))

                def osl(hi, k):
                    return slice((2 * hi + k) * 64, (2 * hi + k + 1) * 64)

                n_out_mm = 8 if snap_a is not None else 6
                mm_i = [0]

                def out_mm(dst, lhsT, rhs):
                    i = mm_i[0]
                    mm_i[0] += 1
                    nc.tensor.matmul(dst, lhsT, rhs, start=(i == 0),
                                     stop=(i == n_out_mm - 1))

                if snap_a is not None:
                    for hi in range(2):
                        out_mm(out_ps[:, osl(hi, 0)], strip(hp, hi, c0),
                               snap_a[:, hi * 64:(hi + 1) * 64])
                for hi in range(2):
                    out_mm(out_ps[:, osl(hi, 1)], strip(hp, hi, c1),
                           snap_b[:, hi * 64:(hi + 1) * 64])
                for k, c in ((0, c0), (1, c1)):
                    for hi in range(2):
                        out_mm(
                            out_ps[:, osl(hi, k)],
                            p_sb[:, (2 * k + hi) * 128:(2 * k + hi + 1) * 128],
                            vsb[(hp, hi)][:, c * N:(c + 1) * N])

                # stage out bf16 and store (one DMA per slot)
                ot = ostp.tile([128, 256], bf, name="ot", tag="ost")
                nc.scalar.copy(ot[:], out_ps[:])
                dst = o.rearrange("p (hh c x) -> p hh c x",
                                  hh=HPC, c=CH // 2)[:, 2 * hp:2 * hp + 2, s]
                src = ot.rearrange("p (h x) -> p h x", h=2)
                nc.sync.dma_start(dst, src)

            # HAM warmup: ~4us of dep-free back-to-back matmuls so the PE
            # clock ungates to 2.4 GHz before the main loop
            wu = wupp.tile([128, 512], f32, name="wu", tag="wu", bufs=1)
            for i in range(12):
                nc.tensor.matmul(wu[:], id_sb[:], cc_sb[:, 0:512],
                                 start=True, stop=True)

            # prologue: transposes for slot 0
            emit_transposes(0, 0)

            for g in range(2 * NSLOT):
                hp, s = g // NSLOT, g % NSLOT
                c0, c1 = 2 * s, 2 * s + 1
                my_snap_a = snap_a_prev[hp]
                if s == 0:
                    m_ps[hp] = mpp.tile([64, 2 * N], f32, name=f"m{hp}",
                                        tag="m", bufs=1)

                # transposes + qrt copy for the NEXT slot (PE + ACT ahead)
                if g + 1 < 2 * NSLOT:
                    nhp, ns = (g + 1) // NSLOT, (g + 1) % NSLOT
                    emit_transposes(nhp, ns)

                # scores for this slot
                p_ps = ppp.tile([128, 512], f32, tag="pp")
                for k, c in ((0, c0), (1, c1)):
                    for hi in range(2):
                        nc.tensor.matmul(
                            p_ps[:, (2 * k + hi) * 128:(2 * k + hi + 1) * 128],
                            strip(hp, hi, c), strip(hp, hi, c),
                            start=True, stop=True)

                # state update with c0 (waits snap_a, copied last slot)
                for hi in range(2):
                    nc.tensor.matmul(
                        m_ps[hp][:, hi * 64:(hi + 1) * 64],
                        qr[(hp, hi)][:, c0 * N:(c0 + 1) * N],
                        vsb[(hp, hi)][:, c0 * N:(c0 + 1) * N],
                        start=(c0 == 0 and hi == 0), stop=False,
                        skip_group_check=True)

                # snapshot M<c1 (after states c0, before states c1)
                snap_b = smallp.tile([64, 2 * N], bf, name="snb", tag="snap")
                nc.scalar.copy(snap_b[:], m_ps[hp][:])

                # B-stage of the previous slot fills PE while snap_b copies
                if g - 1 in pend:
                    emit_B(g - 1)

                # state update with c1 (skip for the last chunk: never read)
                if c1 < CH - 1:
                    for hi in range(2):
                        nc.tensor.matmul(
                            m_ps[hp][:, hi * 64:(hi + 1) * 64],
                            qr[(hp, hi)][:, c1 * N:(c1 + 1) * N],
                            vsb[(hp, hi)][:, c1 * N:(c1 + 1) * N],
                            start=False, stop=(c1 == CH - 2),
                            skip_group_check=True)

                # snapshot M<c0 of the NEXT slot (same hp only)
                snap_a = None
                if s + 1 < NSLOT:
                    snap_a = smallp.tile([64, 2 * N], bf, name="sna",
                                         tag="snap")
                    nc.scalar.copy(snap_a[:], m_ps[hp][:])

                # bridge PSUM -> SBUF with the strict mask folded in
                p_sb = pbp.tile([128, 512], bf, name="psb", tag="psb")
                nc.vector.tensor_mul(p_sb[:], p_ps[:], mu_sb[:])

                # record for B-stage next slot
                out_ps = opp.tile([128, 256], f32, name="outp", tag="outp")
                snaps[(hp, s)] = (my_snap_a, snap_b)
                snap_a_prev[hp] = snap_a
                pend[g] = (hp, s, p_sb, out_ps)

                # warm-keeper: one dep-free wide matmul per slot keeps the
                # HAM activity monitor from re-gating the PE to 1.2 GHz
                nc.tensor.matmul(wu[:], id_sb[:], cc_sb[:, 0:512],
                                 start=True, stop=True)

                for (rhp, st) in rope_sched.get(g, []):
                    emit_rope(rhp, 0, st)
                    emit_rope(rhp, 1, st)

            emit_B(2 * NSLOT - 1)

    nc.compile()
    return nc


_CACHE = {}


def _get_program():
    if "nc" not in _CACHE:
        _CACHE["nc"] = build_program()
    return _CACHE["nc"]


def _in_maps(Q, V):
    Q = np.ascontiguousarray(np.asarray(Q), dtype=np.float32)
    V = np.ascontiguousarray(np.asarray(V), dtype=np.float32)
    qf = Q.reshape(NCORES, HPC, T, N)
    vf = V.reshape(NCORES, HPC, T, N)
    cc2, ss2 = _host_tables()
    mu = _mask_mu()
    ident = _identity()
    return [
        {"q": qf[i], "v": vf[i], "cc2": cc2, "ss2": ss2, "mu": mu,
         "ident": ident}
        for i in range(NCORES)
    ]


def kernel(Q, V):
    from concourse.bass_utils import run_bass_kernel_spmd

    nc = _get_program()
    res = run_bass_kernel_spmd(nc, _in_maps(Q, V),
                               core_ids=list(range(NCORES)))
    # o: [128, HPC, CH, N] bf16, partition-major -> [HPC, T, N] fp32
    out = np.stack(
        [np.asarray(r["o"]).astype(np.float32)
         .reshape(128, HPC, CH, N).transpose(1, 2, 0, 3)
         .reshape(HPC, T, N)
         for r in res.results], axis=0)
    return out.reshape(B, H, T, N)
